# revision 66
# baseline (speedup 1.0000x reference)
"""Trainium2 Bass kernel for LocalDownsampleFlexAttn (24-head attention with
pooled-KV augmentation), head-parallel across 8 NeuronCores.

Sharding: each core owns 3 of the 24 heads. Per core:
  - QKV projections for its 3 heads (column-sliced Wq/Wk/Wv)
  - KV downsampling (4x4 spatial pooling of the 1024 image tokens -> 64)
  - attention over 1536+64 keys
  - partial output projection (row-sliced Wo); host sums the 8 partials + bo.

Design notes (v2):
  - All PE operands fp16 (same PE throughput as bf16, 8x mantissa).
  - Host pre-transposes x and pre-tiles all weights into [128, ...] fp16
    layouts so there are no on-chip transposes or casts.
  - Scores are computed transposed ([key, query]); softmax denominators via
    incremental DVE adds over key-tiles + a GPSIMD partition_all_reduce
    (which also broadcasts), so no PE/ACT cycles are spent on normalization.
  - Emission is software-pipelined: v-projection is woven with head-0/1
    scores; attention rounds weave scores (ACT-paced) against PV +
    output-projection chunks (PE-paced) so the in-order PE stays busy.
"""

import os
import numpy as np
from contextlib import ExitStack

# defensively ask the runtime to reset cores at init: back-to-back runs on
# the shared device occasionally leave it wedged (NRT_EXEC_UNIT_UNRECOVERABLE)
os.environ.setdefault("NEURON_RT_RESET_CORES", "1")

# ---- problem constants (hardcoded per harness contract) ----
S = 1536          # sequence length
DM = 3072         # model dim
NH = 24           # total heads
HD = 128          # head dim
NCORES = 8
HPC = NH // NCORES   # heads per core = 3
CW = HPC * HD        # per-core slice width = 384
TXT = 512
IMG = 1024        # image tokens (32x32)
F = 4             # pooling factor
PK = IMG // (F * F)   # pooled keys = 64
KALL = S + PK     # 1600 keys
NKT = DM // 128   # 24 model-dim k-tiles
NTT = S // 128    # 12 token tiles
NIT = IMG // 128  # 8 image-token tiles
NKC = (KALL + 127) // 128   # 13 key tiles (last has 64)
NQG = S // 512    # 3 query groups
ASCALE = float((1.0 / HD) ** 0.5)

_CACHE = {}


def _weave(a, b):
    """Interleave two lists of emission thunks proportionally (a is paced)."""
    if not b:
        for f in a:
            f()
        return
    if not a:
        for f in b:
            f()
        return
    na, nb = len(a), len(b)
    ia = ib = 0
    while ia < na or ib < nb:
        # keep b ahead proportionally
        if ib * na <= ia * nb and ib < nb:
            b[ib]()
            ib += 1
        elif ia < na:
            a[ia]()
            ia += 1
        else:
            b[ib]()
            ib += 1


def _build_program():
    import concourse.bass as bass
    import concourse.bacc as bacc
    import concourse.tile as tile
    from concourse import bass_isa, mybir

    f32 = mybir.dt.float32
    f16 = mybir.dt.float16
    AF = mybir.ActivationFunctionType
    AX = mybir.AxisListType

    nc = bacc.Bacc(
        "TRN2",
        target_bir_lowering=False,
        debug=False,
        enable_asserts=False,
        num_devices=NCORES,
    )

    xT_d = nc.dram_tensor("xt", [128, NKT * S], f16, kind="ExternalInput").ap()
    wq_d = nc.dram_tensor("wq", [128, NKT * CW], f16, kind="ExternalInput").ap()
    wk_d = nc.dram_tensor("wk", [128, NKT * CW], f16, kind="ExternalInput").ap()
    wv_d = nc.dram_tensor("wv", [128, NKT * CW], f16, kind="ExternalInput").ap()
    wo_d = nc.dram_tensor("wo", [128, HPC * DM], f16, kind="ExternalInput").ap()
    pm_d = nc.dram_tensor("pm", [128, NIT * PK], f16, kind="ExternalInput").ap()
    wf_d = nc.dram_tensor("wf", [IMG], f16, kind="ExternalInput").ap()
    bq_d = nc.dram_tensor("bq", [CW], f32, kind="ExternalInput").ap()
    bk_d = nc.dram_tensor("bk", [CW], f32, kind="ExternalInput").ap()
    bv_d = nc.dram_tensor("bv", [128, CW], f16, kind="ExternalInput").ap()
    out_d = nc.dram_tensor("out", [S, DM], f16, kind="ExternalOutput").ap()

    with tile.TileContext(nc) as tc, ExitStack() as ctx:
        ctx.enter_context(
            nc.allow_low_precision(reason="fp16 softmax sums / pooled keys")
        )
        persist = ctx.enter_context(tc.tile_pool(name="persist", bufs=1))
        psum = ctx.enter_context(tc.tile_pool(name="psum", bufs=1, space="PSUM"))

        # ---- persistent tiles ----
        qT = persist.tile([128, HPC, S], f16)            # q^T per head [d, tok]
        kT = persist.tile([128, HPC, NKC * 128], f16)    # k_all^T per head [d, key]
        vA = persist.tile([128, NKC, CW], f16)           # v_all [key, d(3 heads)]
        attnT = persist.tile([128, HPC, S], f16)         # attn^T [d, tok]
        pm_sb = persist.tile([128, NIT, PK], f16)
        wf_sb = persist.tile([128, IMG], f16)
        bq_sb = persist.tile([128, HPC], f32)
        bk_sb = persist.tile([128, HPC], f32)
        bvb = persist.tile([128, CW], f16)               # bv broadcast to all rows
        tmpw = persist.tile([128, IMG], f16)             # pooled-k scratch
        # early attention-unit tiles (head 0/1, qg 0) live in persist so the
        # phase pools do not have to coexist with xT/weights
        probs_early = [persist.tile([128, NKC, 512], f16, name=f"probs_e{h}")
                       for h in range(2)]
        sumacc_early = [persist.tile([128, 512], f16, name=f"sumacc_e{h}")
                        for h in range(2)]



        # state shared between emission closures
        probs_tiles = {}   # (h, qg) -> (probs_tile, sumacc_tile)

        def emit_exp_sum(h, qg, c, psc, probs, sumacc):
            cs = 128 if c < NKC - 1 else PK
            nc.scalar.activation(
                probs[:cs, c, :], psc[:cs, :], AF.Exp, bias=0.0, scale=ASCALE
            )
            if c == 0:
                nc.vector.tensor_copy(sumacc, probs[:, 0, :])
            elif c < NKC - 1:
                nc.vector.tensor_add(sumacc, sumacc, probs[:, c, :])
            else:
                nc.vector.tensor_add(
                    sumacc[:PK, :], sumacc[:PK, :], probs[:PK, c, :]
                )

        def sc_thunks(h, qg, probs, sumacc):
            """13 thunks: one score matmul + exp + sum-add per key tile."""
            qsl = slice(qg * 512, (qg + 1) * 512)
            thunks = []

            def mk(c):
                def f():
                    cs = 128 if c < NKC - 1 else PK
                    psc = psum.tile([128, 512], f32, tag="sc", bufs=3, name="psc")
                    nc.tensor.matmul(
                        psc[:cs, :],
                        kT[:, h, c * 128:c * 128 + cs],
                        qT[:, h, qsl],
                        start=True,
                        stop=True,
                    )
                    emit_exp_sum(h, qg, c, psc, probs, sumacc)
                return f

            for c in range(NKC):
                thunks.append(mk(c))
            probs_tiles[(h, qg)] = (probs, sumacc)
            return thunks

        def pv_thunks(h, qg, attn_pool):
            """denominator partition-all-reduce (idle GPSIMD engine) +
            13 PV matmuls + normalize."""
            qsl = slice(qg * 512, (qg + 1) * 512)
            rsb_box = []

            def f_denom():
                probs, sumacc = probs_tiles[(h, qg)]
                denom = attn_pool.tile([128, 512], f32, tag="denom", bufs=3,
                                       name="denom")
                nc.gpsimd.partition_all_reduce(
                    denom, sumacc, 128, bass_isa.ReduceOp.add
                )
                rsb = attn_pool.tile([128, 512], f16, tag="rsb", bufs=4, name="rsb")
                nc.vector.reciprocal(rsb, denom)
                rsb_box.append(rsb)

            ppv_box = []

            def mk_pv(c):
                def f():
                    probs, _ = probs_tiles[(h, qg)]
                    cs = 128 if c < NKC - 1 else PK
                    if c == 0:
                        ppv_box.append(
                            psum.tile([128, 512], f32, tag="pv", bufs=2, name="ppv")
                        )
                    nc.tensor.matmul(
                        ppv_box[0],
                        vA[:cs, c, h * 128:(h + 1) * 128],
                        probs[:cs, c, :],
                        start=(c == 0),
                        stop=(c == NKC - 1),
                    )
                return f

            def f_norm():
                nc.vector.tensor_mul(attnT[:, h, qsl], ppv_box[0], rsb_box[0])

            return [f_denom] + [mk_pv(c) for c in range(NKC)] + [f_norm]

        _oeng = [0]
        wo_box = []

        def out_thunks(qt, attn_pool):
            """Output projection for one 128-token tile: 2 halves x 3 col groups."""
            osb_box = [None]

            def mk(cg):
                def f():
                    wo_sb = wo_box[0]
                    if cg % 3 == 0:
                        osb_box[0] = attn_pool.tile(
                            [128, DM // 2], f16, tag="osb", bufs=4, name="osb"
                        )
                    osb = osb_box[0]
                    acc = psum.tile([128, 512], f32, tag="acc", bufs=3, name="oacc")
                    for kt in range(HPC):
                        nc.tensor.matmul(
                            acc,
                            attnT[:, kt, qt * 128:(qt + 1) * 128],
                            wo_sb[:, kt, cg * 512:(cg + 1) * 512],
                            start=(kt == 0),
                            stop=(kt == HPC - 1),
                        )
                    # late tiles (epilogue) have no exp work left: split copies
                    # evenly with ACT; earlier tiles spread across DVE/ACT and
                    # the otherwise-idle Pool engine
                    col = (cg % 3) * 512
                    if qt >= 8:
                        _oeng[0] = (_oeng[0] + 1) % 2
                        if _oeng[0] == 0:
                            nc.scalar.copy(osb[:, col:col + 512], acc)
                        else:
                            nc.vector.tensor_copy(osb[:, col:col + 512], acc)
                    else:
                        _oeng[0] = (_oeng[0] + 1) % 3
                        if _oeng[0] == 0:
                            nc.scalar.copy(osb[:, col:col + 512], acc)
                        else:
                            nc.vector.tensor_copy(osb[:, col:col + 512], acc)
                    if qt >= 11:
                        # kernel tail: per-chunk DMA so the drain after the
                        # last matmul is as short as possible
                        nc.sync.dma_start(
                            out=out_d[qt * 128:(qt + 1) * 128,
                                      cg * 512:(cg + 1) * 512],
                            in_=osb[:, col:col + 512],
                        )
                    elif cg % 3 == 2:
                        g = cg // 3
                        nc.sync.dma_start(
                            out=out_d[qt * 128:(qt + 1) * 128,
                                      g * (DM // 2):(g + 1) * (DM // 2)],
                            in_=osb,
                        )
                return f

            return [mk(cg) for cg in range(6)]

        def emit_pooled_k(h):
            """pooled keys kT[:, h, S:S+PK] from image-token keys (DVE)."""
            for R in range(8):
                nc.vector.tensor_mul(
                    tmpw[:, R * 128:(R + 1) * 128].rearrange(
                        "p (C i j) -> p C i j", C=8, i=4),
                    kT[:, h, TXT + R * 128:TXT + (R + 1) * 128].rearrange(
                        "p (i C j) -> p C i j", i=4, C=8),
                    wf_sb[:, R * 128:(R + 1) * 128].rearrange(
                        "p (i C j) -> p C i j", i=4, C=8),
                )
            nc.vector.reduce_sum(
                kT[:, h, S:S + PK],
                tmpw.rearrange("p (rc ij) -> p rc ij", ij=F * F),
                axis=AX.X,
            )

        # ================= phase 1: projections =================
        with tc.tile_pool(name="xw", bufs=1) as xw:
            xT = xw.tile([128, NKT, S], f16)
            wq_sb = xw.tile([128, NKT, CW], f16)
            wk_sb = xw.tile([128, NKT, CW], f16)
            wv_sb = xw.tile([128, NKT, CW], f16)

            # DMA issue order: first xT/wq tile pair (unblocks the very first
            # matmuls), then the tiny tensors (biases feed the first
            # PSUM->SBUF copies at ~30us), then the xT/wq stream.
            nc.sync.dma_start(out=wq_sb[:, 0, :], in_=wq_d[:, :CW])
            nc.sync.dma_start(out=xT[:, 0, :512], in_=xT_d[:, :512])
            nc.sync.dma_start(out=xT[:, 0, 512:], in_=xT_d[:, 512:S])
            for kt in range(1, NKT):
                nc.sync.dma_start(out=xT[:, kt, :], in_=xT_d[:, kt * S:(kt + 1) * S])
                nc.sync.dma_start(
                    out=wq_sb[:, kt, :], in_=wq_d[:, kt * CW:(kt + 1) * CW]
                )
                if kt == 12:
                    # small tensors: needed from ~t=40us (bias copies), so
                    # keep them out of the startup-critical DMA stream
                    nc.sync.dma_start(
                        out=bq_sb,
                        in_=bass.AP(tensor=bq_d.tensor, offset=0,
                                    ap=[[1, 128], [128, HPC]]),
                    )
                    nc.sync.dma_start(
                        out=bk_sb,
                        in_=bass.AP(tensor=bk_d.tensor, offset=0,
                                    ap=[[1, 128], [128, HPC]]),
                    )
                    nc.sync.dma_start(out=bvb, in_=bv_d)
                    nc.sync.dma_start(
                        out=pm_sb.rearrange("p a b -> p (a b)"), in_=pm_d)
                    nc.sync.dma_start(
                        out=wf_sb,
                        in_=bass.AP(tensor=wf_d.tensor, offset=0,
                                    ap=[[0, 128], [1, IMG]]),
                    )
            nc.sync.dma_start(
                out=wk_sb.rearrange("p a b -> p (a b)"), in_=wk_d)
            nc.sync.dma_start(
                out=wv_sb.rearrange("p a b -> p (a b)"), in_=wv_d)

            # --- q projections: heads 0+1 woven per-kt (absorbs xT DMA), then
            # head 2; then k projections (wk DMA has landed by then) ---
            def proj_finish(accs, b_sb, dstT, h):
                for c in range(3):
                    nc.scalar.activation(
                        dstT[:, h, c * 512:(c + 1) * 512],
                        accs[c],
                        AF.Identity,
                        bias=b_sb[:, h:h + 1],
                        scale=1.0,
                    )

            acc0 = [psum.tile([128, 512], f32, tag="acc", bufs=3, name=f"q0_{c}")
                    for c in range(3)]
            acc1 = [psum.tile([128, 512], f32, tag="sc", bufs=3, name=f"q1_{c}")
                    for c in range(3)]
            # q2's first two column chunks ride along in the spare pv slots so
            # per-kt PE work (8x512 cols) exceeds the xT/wq DMA cadence
            acc2 = [psum.tile([128, 512], f32, tag="pv", bufs=2, name=f"q2_{c}")
                    for c in range(2)]
            for kt in range(NKT):
                for accs, h, nch in ((acc0, 0, 3), (acc1, 1, 3), (acc2, 2, 2)):
                    for c in range(nch):
                        nc.tensor.matmul(
                            accs[c],
                            wq_sb[:, kt, h * 128:(h + 1) * 128],
                            xT[:, kt, c * 512:(c + 1) * 512],
                            start=(kt == 0),
                            stop=(kt == NKT - 1),
                        )
            proj_finish(acc0, bq_sb, qT, 0)
            proj_finish(acc1, bq_sb, qT, 1)
            for c in range(2):
                nc.scalar.activation(
                    qT[:, 2, c * 512:(c + 1) * 512], acc2[c], AF.Identity,
                    bias=bq_sb[:, 2:3], scale=1.0,
                )

            def proj_chunks(w_sb, b_sb, dstT, h, tag, chunks):
                accs = {
                    c: psum.tile([128, 512], f32, tag=tag, bufs=3, name=f"p{h}_{c}")
                    for c in chunks
                }
                for kt in range(NKT):
                    for c in chunks:
                        nc.tensor.matmul(
                            accs[c],
                            w_sb[:, kt, h * 128:(h + 1) * 128],
                            xT[:, kt, c * 512:(c + 1) * 512],
                            start=(kt == 0),
                            stop=(kt == NKT - 1),
                        )
                for c in chunks:
                    nc.scalar.activation(
                        dstT[:, h, c * 512:(c + 1) * 512], accs[c], AF.Identity,
                        bias=b_sb[:, h:h + 1], scale=1.0,
                    )

            proj_chunks(wq_sb, bq_sb, qT, 2, "acc", [2])
            for h in range(HPC):
                proj_chunks(wk_sb, bk_sb, kT, h, "acc" if h % 2 == 0 else "sc",
                            [0, 1, 2])
                emit_pooled_k(h)

            # --- v projection woven with head-0/1 scores (qg 0) ---
            def v_thunk(tt):
                def f():
                    acc = psum.tile([128, 512], f32, tag="acc", bufs=3, name="vacc")
                    for kt in range(NKT):
                        nc.tensor.matmul(
                            acc[:, :CW],
                            xT[:, kt, tt * 128:(tt + 1) * 128],
                            wv_sb[:, kt, :],
                            start=(kt == 0),
                            stop=(kt == NKT - 1),
                        )
                    # bv add fused into the PSUM drain (bv broadcast tile)
                    nc.vector.tensor_add(vA[:, tt, :], acc[:, :CW], bvb)
                return f

            sc00 = sc_thunks(0, 0, probs_early[0], sumacc_early[0])
            sc10 = sc_thunks(1, 0, probs_early[1], sumacc_early[1])
            _weave([v_thunk(tt) for tt in range(NTT)], sc00 + sc10)

            # pooled v (PE), all 3 heads per matmul
            acc = psum.tile([128, 512], f32, tag="acc", bufs=3, name="pvacc")
            for it in range(NIT):
                nc.tensor.matmul(
                    acc[:PK, :CW],
                    pm_sb[:, it, :],
                    vA[:, (TXT // 128) + it, :],
                    start=(it == 0),
                    stop=(it == NIT - 1),
                )
            nc.vector.tensor_copy(vA[:PK, NKC - 1, :], acc[:PK, :CW])

        # ================= phase 2: attention + output =================
        with tc.tile_pool(name="attn", bufs=1) as ap_:
            wo_sb = ap_.tile([128, HPC, DM], f16, tag="wo", name="wo_sb")
            wo_box.append(wo_sb)
            nc.sync.dma_start(
                out=wo_sb.rearrange("p a b -> p (a b)"), in_=wo_d)

            def sc_late(h, qg):
                probs = ap_.tile([128, NKC, 512], f16, tag="probs", bufs=6,
                                 name=f"probs{h}{qg}")
                sumacc = ap_.tile([128, 512], f16, tag="sumacc", bufs=6,
                                  name=f"sumacc{h}{qg}")
                return sc_thunks(h, qg, probs, sumacc)

            # round A: sc(2,0), sc(0,1) vs pv(0,0), pv(1,0), pv(2,0)
            _weave(sc_late(2, 0) + sc_late(0, 1),
                   pv_thunks(0, 0, ap_) + pv_thunks(1, 0, ap_)
                   + pv_thunks(2, 0, ap_))
            # round B: sc(1,1), sc(2,1) vs out qt0..2, pv(0,1)
            _weave(sc_late(1, 1) + sc_late(2, 1),
                   out_thunks(0, ap_) + pv_thunks(0, 1, ap_)
                   + out_thunks(1, ap_) + out_thunks(2, ap_))
            # round C: sc(0,2), sc(1,2) vs pv(1,1), pv(2,1), out qt3..5
            _weave(sc_late(0, 2) + sc_late(1, 2),
                   pv_thunks(1, 1, ap_) + pv_thunks(2, 1, ap_)
                   + out_thunks(3, ap_)
                   + out_thunks(4, ap_) + out_thunks(5, ap_))
            # round D: sc(2,2) vs pv(0,2), pv(1,2), out qt6, qt7; then pv(2,2)
            _weave(sc_late(2, 2),
                   pv_thunks(0, 2, ap_) + pv_thunks(1, 2, ap_)
                   + out_thunks(6, ap_) + out_thunks(7, ap_))
            for f in pv_thunks(2, 2, ap_):
                f()
            # epilogue: pure output projection
            for qt in range(8, 12):
                for f in out_thunks(qt, ap_):
                    f()

    nc.compile()
    return nc


def _get_program():
    if "nc" not in _CACHE:
        _CACHE["nc"] = _build_program()
    return _CACHE["nc"]


def _prep_in_maps(hidden_states, Wq, bq, Wk, bk, Wv, bv, Wo, spatial_weight):
    f16 = np.float16
    x = np.asarray(hidden_states, dtype=np.float32).reshape(S, DM)
    # x^T tiled: [p, kt*S + t] = x[t, kt*128 + p]
    xT = np.ascontiguousarray(
        x.reshape(S, NKT, 128).transpose(2, 1, 0).reshape(128, NKT * S)
    ).astype(f16)

    Wq = np.asarray(Wq, dtype=np.float32)
    Wk = np.asarray(Wk, dtype=np.float32)
    Wv = np.asarray(Wv, dtype=np.float32)
    Wo = np.asarray(Wo, dtype=np.float32)
    bq = np.asarray(bq, dtype=np.float32)
    bk = np.asarray(bk, dtype=np.float32)
    bv = np.asarray(bv, dtype=np.float32)

    w = np.asarray(spatial_weight, dtype=np.float32).reshape(F, F)  # [i, j]
    # wfull[t] for t = 128R + 32i + 4C + j  -> broadcast w over (R, C)
    wfull = np.ascontiguousarray(
        np.broadcast_to(w[None, :, None, :], (8, F, 8, F)).reshape(IMG)
    ).astype(f16)
    # pmat[t, R*8+C] = w[i, j] for t in block (R, C), tiled [p, it*PK + j]
    pmat = np.zeros((8, F, 8, F, 8, 8), dtype=np.float32)
    for R in range(8):
        for C in range(8):
            pmat[R, :, C, :, R, C] = w
    pmat = pmat.reshape(IMG, PK)
    pm_t = np.ascontiguousarray(
        pmat.reshape(NIT, 128, PK).transpose(1, 0, 2).reshape(128, NIT * PK)
    ).astype(f16)

    def tile_w(Wslice):  # [DM, CW] -> [128, NKT*CW]
        return np.ascontiguousarray(
            Wslice.reshape(NKT, 128, CW).transpose(1, 0, 2).reshape(128, NKT * CW)
        ).astype(f16)

    in_maps = []
    for c in range(NCORES):
        sl = slice(c * CW, (c + 1) * CW)
        wo_slice = Wo[sl, :]  # [CW, DM]
        wo_t = np.ascontiguousarray(
            wo_slice.reshape(HPC, 128, DM).transpose(1, 0, 2).reshape(128, HPC * DM)
        ).astype(f16)
        in_maps.append({
            "xt": xT,
            "wq": tile_w(Wq[:, sl]),
            "wk": tile_w(Wk[:, sl]),
            "wv": tile_w(Wv[:, sl]),
            "wo": wo_t,
            "pm": pm_t,
            "wf": wfull,
            "bq": np.ascontiguousarray(bq[sl]),
            "bk": np.ascontiguousarray(bk[sl]),
            "bv": np.ascontiguousarray(
                np.broadcast_to(bv[sl].astype(f16)[None, :], (128, CW))
            ),
        })
    return in_maps


def _run(inputs, trace=False, trace_kwargs=None):
    from concourse import bass_utils

    nc = _get_program()
    in_maps = _prep_in_maps(
        inputs["hidden_states"], inputs["Wq"], inputs["bq"], inputs["Wk"],
        inputs["bk"], inputs["Wv"], inputs["bv"], inputs["Wo"],
        inputs["spatial_weight"],
    )
    res = None
    last_err = None
    for _attempt in range(2):
        try:
            res = bass_utils.run_bass_kernel_spmd(
                nc, in_maps, list(range(NCORES)), trace=trace,
                **(trace_kwargs or {}),
            )
            break
        except Exception as e:  # transient device wedge: retry
            last_err = e
    if res is None:
        raise last_err
    partial = np.zeros((S, DM), dtype=np.float32)
    for r in res.results:
        partial += r["out"].astype(np.float32)
    out = partial + np.asarray(inputs["bo"], dtype=np.float32)[None, :]
    return out.reshape(1, S, DM).astype(np.float32), res


def _kernel_subprocess_retry(inputs):
    """Re-run in a fresh process: a wedged NeuronCore session
    (NRT_EXEC_UNIT_UNRECOVERABLE) empirically clears for the next process."""
    import subprocess
    import sys
    import tempfile

    kdir = os.path.dirname(os.path.abspath(__file__))
    with tempfile.TemporaryDirectory() as td:
        np.savez(
            os.path.join(td, "in.npz"),
            **{k: np.asarray(v) for k, v in inputs.items()},
        )
        script = (
            "import os, sys, numpy as np\n"
            "os.environ['_KERNEL_SUBPROC'] = '1'\n"
            f"sys.path.insert(0, {kdir!r})\n"
            "import kernel\n"
            f"d = np.load(os.path.join({td!r}, 'in.npz'))\n"
            "inp = {k: d[k] for k in d.files}\n"
            "out = kernel.kernel(**inp)\n"
            f"np.save(os.path.join({td!r}, 'out.npy'), out)\n"
        )
        r = subprocess.run([sys.executable, "-c", script], capture_output=True)
        outp = os.path.join(td, "out.npy")
        if not os.path.exists(outp):
            sys.stderr.write(r.stdout.decode()[-2000:])
            sys.stderr.write(r.stderr.decode()[-2000:])
            return None
        return np.load(outp)


def kernel(**inputs):
    h = int(inputs.get("height", 32))
    w = int(inputs.get("width", 32))
    assert h == 32 and w == 32, (h, w)
    try:
        out, _ = _run(inputs, trace=False)
        return out
    except Exception:
        if os.environ.get("_KERNEL_SUBPROC") == "1":
            raise
        for _ in range(2):
            out = _kernel_subprocess_retry(inputs)
            if out is not None:
                return out
        raise
